# revision 11
# baseline (speedup 1.0000x reference)
"""AudioVQVAE forward pass on 8 Trainium2 NeuronCores (Bass/Tile).

Self-contained: `kernel(**inputs) -> (rec, com, ppx, xr, idx)`.

Sharding: time-parallel. core = 2*b + h handles batch b, half h of the
time axis (4096 z-positions + halo). No collectives; halos are carved on
the host from the full inputs. Scalars (rec/com) are reduced on device to
per-core partial sums, finished on host; ppx depends only on the tiny
cluster_size input and is computed on host.

Per-core device pipeline (all conv stages are PE matmuls):
  conv1 (K=8 tap contraction) -> gelu -> conv2 (8 accumulated taps,
  stride-4 rhs) -> gelu -> conv3 -> z [128, 4224]
  VQ: 33 pos-tiles: sim = z_tile.T @ embTn  [128, 2048] in PSUM;
      DVE reduce_max + custom DVE op (select(x>=max, -Idx, -FLT_MAX),
      accum=MAX) = exact first-index argmax; gpsimd indirect DMA gathers
      embedding rows; PE transpose -> z_qT.
  decoder: conv0 -> gelu -> convT1 (phase decomposition, 4 phases x 2
  taps) -> gelu -> convT2 (A/B tap matmuls packed 4 q-chunks per PSUM
  bank) -> tanh -> xr, plus on-device (z-z_q)^2 and (xr-x)^2 partials.
"""

import numpy as np

# ---------------------------------------------------------------- geometry
B, T, D, KCB = 4, 131072, 128, 2048
HALO = 64
NPOS = 4096 + 2 * HALO          # 4224 z positions per core
NT = NPOS // 128                # 33 pos-tiles
L2 = NPOS + 2                   # y2 length
L1 = 4 * L2 + 4                 # 16908 y1 positions
L1P = 16912
LX = 4 * L1P + 8                # x slab length 67656
NCHUNK = 9                      # z chunks: 8x512 + 1x128
ZC = [512] * 8 + [128]
N_DEC = 8                       # decoder chunks of 2048 q each

F32 = None  # set at bass import time


# ---------------------------------------------------------------- host prep
def _host_prepare(inputs):
    f = np.float32
    w = {}
    w1 = np.asarray(inputs['enc_w1'], f)
    w['w1T'] = np.ascontiguousarray(w1[:, 0, :].T)                    # [8,128]
    for src, dst in (('enc_w2', 'w2T'), ('enc_w3', 'w3T'), ('dec_w0', 'w0T')):
        a = np.asarray(inputs[src], f)                                # [out,in,k]
        w[dst] = np.ascontiguousarray(np.transpose(a, (1, 2, 0)))     # [in,k,out]
    wt1 = np.asarray(inputs['dec_wt1'], f)                            # [in,out,8]
    ab = np.zeros((D, 8, D), f)                                       # [in, 8, out]
    for r in range(4):
        ab[:, r, :] = wt1[:, :, 2 + r]                                # A_r
        ab[:, 4 + r, :] = wt1[:, :, 6 + r] if r < 2 else wt1[:, :, r - 2]  # B_r
    w['wt1AB'] = ab
    wt2 = np.asarray(inputs['dec_wt2'], f)                            # [in,1,8]
    w2ab = np.zeros((D, 12), f)   # cols 0:4 A, 4:8 B01, 8:12 B23
    for r in range(4):
        w2ab[:, r] = wt2[:, 0, 2 + r]
    w2ab[:, 4] = wt2[:, 0, 6]
    w2ab[:, 5] = wt2[:, 0, 7]
    w2ab[:, 10] = wt2[:, 0, 0]
    w2ab[:, 11] = wt2[:, 0, 1]
    w['wt2AB'] = w2ab
    bias = np.zeros((D, 6), f)
    bias[:, 0] = np.asarray(inputs['enc_b1'], f)
    bias[:, 1] = np.asarray(inputs['enc_b2'], f)
    bias[:, 2] = np.asarray(inputs['enc_b3'], f)
    bias[:, 3] = np.asarray(inputs['dec_b0'], f)
    bias[:, 4] = np.asarray(inputs['dec_bt1'], f)
    bias[:, 5] = np.asarray(inputs['dec_bt2'], f)[0]
    w['biases'] = bias
    masks = np.ones((D, 2), f)                                        # ones vectors
    w['masks'] = masks
    emb = np.ascontiguousarray(np.asarray(inputs['embedding'], f))
    w['embedding'] = emb
    nrm = np.sqrt(np.sum(emb.astype(f) * emb, axis=-1, dtype=f)).astype(f)
    nrm = np.maximum(nrm, f(1e-12))
    w['embn'] = (emb / nrm[:, None]).astype(f)
    return w


def _core_slabs(x, core):
    h = core % 2
    b = core // 2
    VQ_LO = 4096 * h - HALO
    t0 = 16 * VQ_LO - 26
    slab = np.zeros(LX, np.float32)
    lo, hi = max(0, t0), min(T, t0 + LX)
    if hi > lo:
        slab[lo - t0:hi - t0] = x[b, 0, lo:hi]
    x8 = np.zeros((8, L1P), np.float32)
    for t in range(8):
        x8[t] = slab[t:t + 4 * L1P:4]
    T0 = 65536 * h
    x4 = np.asarray(x[b, 0, T0:T0 + 65536], np.float32).reshape(16384, 4).T
    x4 = np.ascontiguousarray(x4)                                     # [4,16384]
    return x8, x4


# ------------------------------------------------------- custom DVE argmax
_ARGMAX_OP = None


def _get_argmax_op():
    global _ARGMAX_OP
    if _ARGMAX_OP is not None:
        return _ARGMAX_OP
    from concourse import dve_ops
    from concourse.dve_spec import (Spec, Src0, C0, Zero, MaxNeg, Idx, AluOp,
                                    lower as dve_lower, select, _has_src1)
    from concourse.dve_uop import DveOpSpec

    name = "ARGMAX_GE_ANT"
    if name in dve_ops._SUB_OPCODE_FOR_NAME:
        _ARGMAX_OP = next(op for op in dve_ops.OPS if op.name == name)
        return _ARGMAX_OP

    def _ref(in0, in1, c0, c1, c2):
        n = in0.shape[-1]
        iota = -np.arange(n, dtype=np.float32)
        out = np.where(in0.reshape(in0.shape[0], -1) >= c0, iota,
                       np.float32(-3.4028235e38))
        return out, out.max(-1, keepdims=True)

    spec = Spec(body=select(Src0 >= C0, Zero - Idx, MaxNeg),
                accum=AluOp.MAX, reference=_ref)
    row = max(dve_ops._SUB_OPCODE_FOR_NAME.values()) + 1
    shas = {}
    for ver in ("v3", "v4"):
        try:
            uops = dve_lower(spec, ver=ver)
            shas[ver] = DveOpSpec(name=name, opcode=row, uops=uops,
                                  rd1_en=_has_src1(spec)).sha(ver)
        except Exception:
            pass
    op = dve_ops.DveOp(name, spec, subdim=False, uops_sha=shas)
    dve_ops.OPS.append(op)
    dve_ops.CUSTOM_DVE_SPECS[name] = spec
    dve_ops._SUB_OPCODE_FOR_NAME[name] = row
    _ARGMAX_OP = op
    return op


# ------------------------------------------------------------ bass builder
_BUILT = None

# float32r streams at full PE rate but is reduced precision and must be
# produced (rounded) by the writing op; float32 is exact but 4 cycles/row.
# Flags flip buffer/weight dtypes per stage once precision is measured.
DEC_F32R = True


def _build():
    global _BUILT, F32
    if _BUILT is not None:
        return _BUILT
    import concourse.bass as bass
    import concourse.mybir as mybir
    import concourse.tile as tile
    from concourse import bacc
    from concourse.masks import make_identity
    from contextlib import ExitStack

    F32 = mybir.dt.float32
    F32R = mybir.dt.float32r
    AF = mybir.ActivationFunctionType
    ALU = mybir.AluOpType
    AX = mybir.AxisListType
    argmax_op = _get_argmax_op()

    nc = bacc.Bacc("TRN2", target_bir_lowering=False, debug=False,
                   enable_asserts=False, num_devices=8)

    dt_i32 = mybir.dt.int32
    # ---- DRAM tensors
    x8_d = nc.dram_tensor("x8", [8, L1P], F32, kind="ExternalInput").ap()
    x4_d = nc.dram_tensor("x4", [4, 16384], F32, kind="ExternalInput").ap()
    w1_d = nc.dram_tensor("w1T", [8, D], F32, kind="ExternalInput").ap()
    w2_d = nc.dram_tensor("w2T", [D, 8, D], F32, kind="ExternalInput").ap()
    w3_d = nc.dram_tensor("w3T", [D, 3, D], F32, kind="ExternalInput").ap()
    w0_d = nc.dram_tensor("w0T", [D, 3, D], F32, kind="ExternalInput").ap()
    wt1_d = nc.dram_tensor("wt1AB", [D, 8, D], F32, kind="ExternalInput").ap()
    wt2_d = nc.dram_tensor("wt2AB", [D, 12], F32, kind="ExternalInput").ap()
    bias_d = nc.dram_tensor("biases", [D, 6], F32, kind="ExternalInput").ap()
    mask_d = nc.dram_tensor("masks", [D, 2], F32, kind="ExternalInput").ap()
    emb_d = nc.dram_tensor("embedding", [KCB, D], F32, kind="ExternalInput").ap()
    embn_d = nc.dram_tensor("embn", [KCB, D], F32, kind="ExternalInput").ap()
    xr_d = nc.dram_tensor("xr_out", [65536], F32, kind="ExternalOutput").ap()
    idx_d = nc.dram_tensor("idx_out", [4096], dt_i32, kind="ExternalOutput").ap()
    part_d = nc.dram_tensor("partials", [2, 2], F32, kind="ExternalOutput").ap()
    zdbg_d = nc.dram_tensor("z_dbg", [D, NPOS], F32, kind="ExternalOutput").ap()
    etdbg_d = nc.dram_tensor("et_dbg", [D, KCB], F32, kind="ExternalOutput").ap()

    ctx = ExitStack()
    with tile.TileContext(nc) as tc, ctx:
        P = ctx.enter_context(tc.tile_pool(name="persist", bufs=1))
        ps = ctx.enter_context(tc.tile_pool(name="ps", bufs=1, space="PSUM"))
        sl = ctx.enter_context(tc.tile_pool(name="slabs", bufs=2))

        # ---- persistent SBUF
        w1sb = P.tile([8, D], F32, tag="w1")
        w2sb = P.tile([D, 8, D], F32, tag="w2")
        w3sb = P.tile([D, 3, D], F32, tag="w3")
        w0sb = P.tile([D, 3, D], F32, tag="w0")
        wt1sb = P.tile([D, 8, D], F32, tag="wt1")
        wt2sb = P.tile([D, 12], F32, tag="wt2")
        biassb = P.tile([D, 6], F32, tag="bias")
        masksb = P.tile([D, 2], F32, tag="mask")
        ident = P.tile([D, D], F32, tag="ident")
        embTn = P.tile([D, KCB], F32, tag="embTn")
        y2 = P.tile([D, L2 + 2], F32, tag="y2")
        z = P.tile([D, NPOS], F32, tag="z")
        zqg = P.tile([D, NT, D], F32, tag="zqg")
        DTD = F32R if DEC_F32R else F32
        zqT = P.tile([D, NPOS], DTD, tag="zqT")
        h0 = P.tile([D, NPOS], DTD, tag="h0")
        idx32 = P.tile([D, NT], dt_i32, tag="idx32")
        parts = P.tile([D, 48], F32, tag="parts")
        nc.gpsimd.memset(parts[:], 0.0)
        pairb = P.tile([D, 2], F32, tag="pairb")
        scr = P.tile([D, KCB], F32, tag="scr")

        for sb_t, d_ap in ((w1sb, w1_d), (w2sb, w2_d), (w3sb, w3_d),
                           (w0sb, w0_d), (wt1sb, wt1_d), (wt2sb, wt2_d),
                           (biassb, bias_d), (masksb, mask_d)):
            nc.sync.dma_start(sb_t[:], d_ap[:])
        make_identity(nc, ident[:])
        w0r = P.tile([D, 3, D], DTD, tag="w0r")
        wt1r = P.tile([D, 8, D], DTD, tag="wt1r")
        wt2r = P.tile([D, 12], DTD, tag="wt2r")
        nc.vector.tensor_copy(w0r[:], w0sb[:])
        nc.vector.tensor_copy(wt1r[:], wt1sb[:])
        nc.vector.tensor_copy(wt2r[:], wt2sb[:])

        def bias_ap(i):
            return biassb[:, i:i + 1]

        # ---- embedding: load host-normalized, transpose -> embTn [D, 2048]
        with tc.tile_pool(name="embinit", bufs=1) as EI:
            embn = EI.tile([D, 16, D], F32, tag="embn")
            nc.sync.dma_start(embn[:], embn_d.rearrange("(t p) d -> p t d", p=D))
            for t in range(16):
                tp = ps.tile([D, D], F32, tag="tp", bufs=1)
                nc.tensor.transpose(tp[:], embn[:, t, :], ident[:])
                nc.scalar.copy(embTn[:, D * t:D * (t + 1)], tp[:])

        # ---------------------------------------------------- phase emitters
        def conv1_slab(c):
            lc2 = min(512, L2 - 512 * c)
            w = 4 * lc2 + 4
            n0 = 2048 * c
            xsl = sl.tile([8, 2052], F32, tag="x8sl")
            nc.sync.dma_start(xsl[:, :w], x8_d[:, n0:n0 + w])
            y1s = sl.tile([D, 2052], F32, tag="y1sl")
            off = 0
            while off < w:
                n = min(512, w - off)
                pm = ps.tile([D, 512], F32, tag="mm", bufs=3)
                nc.tensor.matmul(pm[:, :n], w1sb[:],
                                 xsl[:, off:off + n],
                                 start=True, stop=True)
                nc.scalar.activation(y1s[:, off:off + n], pm[:, :n], AF.Gelu,
                                     bias=bias_ap(0))
                off += n
            return y1s

        def conv2_chunk(c, y1s):
            lc2 = min(512, L2 - 512 * c)
            pm = ps.tile([D, 512], F32, tag="mm", bufs=3)
            v3 = y1s[:].rearrange("p (n four) -> p n four", four=4)
            for t in range(8):
                rhs = v3[:, t // 4: t // 4 + lc2, t % 4]
                nc.tensor.matmul(pm[:, :lc2], w2sb[:, t, :], rhs,
                                 start=(t == 0), stop=(t == 7))
            nc.scalar.activation(y2[:, 512 * c:512 * c + lc2], pm[:, :lc2],
                                 AF.Gelu, bias=bias_ap(1))

        def conv3_chunk(c):
            zc = ZC[c]
            pm = ps.tile([D, 512], F32, tag="mm", bufs=3)
            for t in range(3):
                rhs = y2[:, 512 * c + t:512 * c + t + zc]
                nc.tensor.matmul(pm[:, :zc], w3sb[:, t, :], rhs,
                                 start=(t == 0), stop=(t == 2))
            nc.scalar.activation(z[:, 512 * c:512 * c + zc], pm[:, :zc],
                                 AF.Identity, bias=bias_ap(2))

        def vq_tile(i):
            lhsT = z[:, D * i:D * (i + 1)]
            mx2 = sl.tile([D, 2], F32, tag="mx2")
            acc2 = sl.tile([D, 2], F32, tag="acc2")
            halves = []
            for hh in range(2):
                sh = ps.tile([D, KCB // 2], F32, tag="sim", bufs=2)
                halves.append(sh)
                for kk in range(2):
                    k0 = 1024 * hh + 512 * kk
                    nc.tensor.matmul(sh[:, 512 * kk:512 * (kk + 1)], lhsT,
                                     embTn[:, k0:k0 + 512],
                                     start=True, stop=True)
                nc.vector.tensor_reduce(mx2[:, hh:hh + 1], sh[:], axis=AX.X,
                                        op=ALU.max)
            mx = sl.tile([D, 1], F32, tag="mx")
            nc.vector.tensor_reduce(mx[:], mx2[:], axis=AX.X, op=ALU.max)
            for hh in range(2):
                nc.vector._custom_dve(argmax_op, out=scr[:, 1024 * hh:1024 * (hh + 1)],
                                      in0=halves[hh][:], s0=mx[:],
                                      accum_out=acc2[:, hh:hh + 1])
            neg = sl.tile([D, 1], F32, tag="neg")
            nc.vector.tensor_scalar(neg[:], acc2[:, 1:2], -1024.0, None,
                                    op0=ALU.add)
            nc.vector.tensor_tensor(out=neg[:], in0=neg[:], in1=acc2[:, 0:1],
                                    op=ALU.max)
            nc.vector.tensor_scalar(idx32[:, i:i + 1], neg[:], -1.0, None,
                                    op0=ALU.mult)
            nc.gpsimd.indirect_dma_start(
                out=zqg[:, i, :], out_offset=None, in_=emb_d[:],
                in_offset=bass.IndirectOffsetOnAxis(ap=idx32[:, i:i + 1], axis=0),
                bounds_check=KCB - 1, oob_is_err=False)
            tp = ps.tile([D, D], F32, tag="tp", bufs=1)
            nc.tensor.transpose(tp[:], zqg[:, i, :], ident[:])
            nc.scalar.copy(zqT[:, D * i:D * (i + 1)], tp[:])

        def com_chunk(r):
            c0 = HALO + 512 * r
            dsc = sl.tile([D, 512], F32, tag="dsc")
            nc.vector.tensor_tensor(out=dsc[:], in0=z[:, c0:c0 + 512],
                                    in1=zqT[:, c0:c0 + 512].bitcast(F32), op=ALU.subtract)
            nc.scalar.activation(dsc[:], dsc[:], AF.Square,
                                 accum_out=parts[:, r:r + 1])

        def conv0_chunk(cc):
            l0 = 62 + 512 * cc
            n = min(512, 4162 - l0)
            if n <= 0:
                return
            pm = ps.tile([D, 512], F32, tag="mm", bufs=3)
            for t in range(3):
                rhs = zqT[:, l0 - 1 + t:l0 - 1 + t + n]
                nc.tensor.matmul(pm[:, :n], w0r[:, t, :], rhs,
                                 start=(t == 0), stop=(t == 2))
            nc.scalar.activation(h0[:, l0:l0 + n], pm[:, :n], AF.Gelu,
                                 bias=bias_ap(3))

        def convT1_chunk(ct):
            """h1 slab [D, 4*514] m-contiguous, m0 = 4*U0, U0 = 63+512*ct."""
            U0 = 63 + 512 * ct
            h1s = sl.tile([D, 4 * 514], DTD, tag="h1sl")
            v3 = h1s[:].rearrange("p (u four) -> p u four", four=4)
            for r in range(4):
                dshift = -1 if r < 2 else 1
                for seg0, segn in ((0, 512), (512, 2)):
                    pm = ps.tile([D, 512], F32, tag="mm", bufs=3)
                    ra = h0[:, U0 + seg0:U0 + seg0 + segn]
                    rb = h0[:, U0 + dshift + seg0:U0 + dshift + seg0 + segn]
                    nc.tensor.matmul(pm[:, :segn], wt1r[:, r, :],
                                     ra, start=True, stop=False)
                    nc.tensor.matmul(pm[:, :segn], wt1r[:, 4 + r, :],
                                     rb, start=False, stop=True)
                    nc.scalar.activation(v3[:, seg0:seg0 + segn, r], pm[:, :segn],
                                         AF.Gelu, bias=bias_ap(4))
            return h1s

        def convT2_chunk(ct, h1s):
            xst = sl.tile([16, 512], F32, tag="xrt")
            xc = sl.tile([16, 512], F32, tag="xcmp")
            nc.sync.dma_start(
                xc[:].rearrange("(s four) n -> four s n", four=4),
                x4_d[:, 2048 * ct:2048 * (ct + 1)].rearrange(
                    "four (s n) -> four s n", s=4))
            for s in range(4):
                base = 4 + 512 * s
                pm = ps.tile([D, 512], F32, tag="mm", bufs=3)
                nc.tensor.matmul(pm[0:4, :], wt2r[:, 0:4],
                                 h1s[:, base:base + 512],
                                 start=True, stop=False)
                nc.tensor.matmul(pm[0:4, :], wt2r[:, 4:8],
                                 h1s[:, base - 1:base + 511],
                                 start=False, stop=False)
                nc.tensor.matmul(pm[0:4, :], wt2r[:, 8:12],
                                 h1s[:, base + 1:base + 513],
                                 start=False, stop=True)
                nc.scalar.activation(xst[4 * s:4 * s + 4, :], pm[0:4, :], AF.Tanh,
                                     bias=biassb[0:4, 5:6])
                o = 8192 * ct + 2048 * s
                nc.sync.dma_start(
                    xr_d[o:o + 2048].rearrange("(i four) -> four i", four=4),
                    xst[4 * s:4 * s + 4, :])
            dsc = sl.tile([16, 512], F32, tag="dsc2")
            nc.vector.tensor_tensor(out=dsc[:], in0=xst[:], in1=xc[:],
                                    op=ALU.subtract)
            nc.scalar.activation(dsc[:], dsc[:], AF.Square,
                                 accum_out=parts[0:16, 9 + ct:10 + ct])

        # ------------------------------------------------------ emission
        # conv3(c) reads 2 y2 columns produced by conv2(c+1); com/conv0 read
        # one z/zqT tile of the next VQ chunk; convT1 reads 3 h0 columns of
        # the next conv0 chunk -- so each stage lags its producer by one
        # chunk in emission order.
        for c in range(NCHUNK + 3):
            if c < NCHUNK:
                y1s = conv1_slab(c)
                conv2_chunk(c, y1s)
            if 1 <= c <= NCHUNK:
                conv3_chunk(c - 1)
                t0 = 4 * (c - 1)
                for i in range(t0, min(NT, t0 + 4)):
                    vq_tile(i)
            if 2 <= c:
                if c - 2 < 8:
                    com_chunk(c - 2)
                if c - 2 <= 8:
                    conv0_chunk(c - 2)
            if 3 <= c and c - 3 < 8:
                h1s = convT1_chunk(c - 3)
                convT2_chunk(c - 3, h1s)

        # ------------------------------------------------------ epilogue
        nc.vector.tensor_reduce(pairb[:, 0:1], parts[:, 9:17], axis=AX.X,
                                op=ALU.add)
        nc.vector.tensor_reduce(pairb[:, 1:2], parts[:, 0:8], axis=AX.X,
                                op=ALU.add)
        pp = ps.tile([2, 2], F32, tag="mm", bufs=3)
        nc.tensor.matmul(pp[:], masksb[:], pairb[:], start=True, stop=True)
        ppsb = P.tile([2, 2], F32, tag="ppsb")
        nc.scalar.copy(ppsb[:], pp[:])
        nc.sync.dma_start(part_d[:], ppsb[:])

        nc.sync.dma_start(zdbg_d[:], z[:])
        nc.sync.dma_start(etdbg_d[:], embTn[:])
        # idx output: interior positions 64..4159
        nc.sync.dma_start(idx_d[0:64], idx32[64:128, 0:1])
        nc.sync.dma_start(idx_d[64:64 + 31 * 128].rearrange("(t p) -> p t", p=128),
                          idx32[:, 1:32])
        nc.sync.dma_start(idx_d[4032:4096], idx32[0:64, 32:33])

    nc.compile()
    _BUILT = nc
    return nc


# ---------------------------------------------------------------- kernel()
LAST_RUN = None


def kernel(**inputs):
    from concourse.bass_utils import run_bass_kernel_spmd
    global LAST_RUN

    w = _host_prepare(inputs)
    x = np.asarray(inputs['x'], np.float32)
    shared = {
        'w1T': w['w1T'], 'w2T': w['w2T'], 'w3T': w['w3T'], 'w0T': w['w0T'],
        'wt1AB': w['wt1AB'], 'wt2AB': w['wt2AB'], 'biases': w['biases'],
        'masks': w['masks'], 'embedding': w['embedding'], 'embn': w['embn'],
    }
    in_maps = []
    for core in range(8):
        x8, x4 = _core_slabs(x, core)
        m = dict(shared)
        m['x8'] = x8
        m['x4'] = x4
        in_maps.append(m)

    nc = _build()
    res = run_bass_kernel_spmd(nc, in_maps, core_ids=list(range(8)))
    LAST_RUN = res

    xr_full = np.zeros((B, 1, T), np.float32)
    idx_full = np.zeros((B, 8192), np.int32)
    rec_sum = 0.0
    com_sum = 0.0
    for core in range(8):
        r = res.results[core]
        b, h = core // 2, core % 2
        xr_full[b, 0, 65536 * h:65536 * (h + 1)] = r['xr_out']
        idx_full[b, 4096 * h:4096 * (h + 1)] = r['idx_out']
        rec_sum += float(r['partials'][0, 0])
        com_sum += float(r['partials'][1, 1])

    rec = np.float32(rec_sum / (B * T))
    com = np.float32(com_sum / (B * D * 8192))
    cs = np.asarray(inputs['cluster_size'], np.float64)
    n = cs / (cs.sum() + 1e-6)
    ppx = np.float32(np.exp(-np.sum(n * np.log(n + 1e-6))))
    return rec, com, ppx, xr_full, idx_full


# revision 20
# speedup vs baseline: 1.9351x; 1.9351x over previous
"""AudioVQVAE forward pass on 8 Trainium2 NeuronCores (Bass/Tile).

Self-contained: `kernel(**inputs) -> (rec, com, ppx, xr, idx)`.

Sharding: time-parallel. core = 2*b + h handles batch b, half h of the
time axis (4096 z-positions + halo). No collectives; halos are carved on
the host from the full inputs. Scalars (rec/com) are reduced on device to
per-core partial sums, finished on host; ppx depends only on the tiny
cluster_size input and is computed on host.

Per-core device pipeline (all conv stages are PE matmuls):
  conv1 (K=8 tap contraction) -> gelu -> conv2 (8 accumulated taps,
  stride-4 rhs) -> gelu -> conv3 -> z [128, 4224]
  VQ: 33 pos-tiles: sim = z_tile.T @ embTn  [128, 2048] in PSUM;
      DVE reduce_max + custom DVE op (select(x>=max, -Idx, -FLT_MAX),
      accum=MAX) = exact first-index argmax; gpsimd indirect DMA gathers
      embedding rows; PE transpose -> z_qT.
  decoder: conv0 -> gelu -> convT1 (phase decomposition, 4 phases x 2
  taps) -> gelu -> convT2 (A/B tap matmuls packed 4 q-chunks per PSUM
  bank) -> tanh -> xr, plus on-device (z-z_q)^2 and (xr-x)^2 partials.
"""

import os

import numpy as np

# ---------------------------------------------------------------- geometry
B, T, D, KCB = 4, 131072, 128, 2048
HALO = 64
NPOS = 4096 + 2 * HALO          # 4224 z positions per core
NT = NPOS // 128                # 33 pos-tiles
L2 = NPOS + 2                   # y2 length
L1 = 4 * L2 + 4                 # 16908 y1 positions
L1P = 16912
LX = 4 * L1P + 8                # x slab length 67656
NCHUNK = 9                      # z chunks: 8x512 + 1x128
ZC = [512] * 8 + [128]
N_DEC = 8                       # decoder chunks of 2048 q each

F32 = None  # set at bass import time


# ---------------------------------------------------------------- host prep
def _host_prepare(inputs):
    f = np.float32
    w = {}
    w1 = np.asarray(inputs['enc_w1'], f)
    w1T = np.ascontiguousarray(w1[:, 0, :].T)                         # [8,128]
    w1rep = np.zeros((D, D), f)
    for g in range(4):
        w1rep[32 * g:32 * g + 8, :] = w1T
    w['w1T'] = w1rep
    for src, dst in (('enc_w2', 'w2T'), ('enc_w3', 'w3T'), ('dec_w0', 'w0T')):
        a = np.asarray(inputs[src], f)                                # [out,in,k]
        w[dst] = np.ascontiguousarray(np.transpose(a, (1, 2, 0)))     # [in,k,out]
    wt1 = np.asarray(inputs['dec_wt1'], f)                            # [in,out,8]
    ab = np.zeros((D, 8, D), f)                                       # [in, 8, out]
    for r in range(4):
        ab[:, r, :] = wt1[:, :, 2 + r]                                # A_r
        ab[:, 4 + r, :] = wt1[:, :, 6 + r] if r < 2 else wt1[:, :, r - 2]  # B_r
    w['wt1AB'] = ab
    wt2 = np.asarray(inputs['dec_wt2'], f)                            # [in,1,8]
    w2ab = np.zeros((D, 12), f)   # cols 0:4 A, 4:8 B01, 8:12 B23
    for r in range(4):
        w2ab[:, r] = wt2[:, 0, 2 + r]
    w2ab[:, 4] = wt2[:, 0, 6]
    w2ab[:, 5] = wt2[:, 0, 7]
    w2ab[:, 10] = wt2[:, 0, 0]
    w2ab[:, 11] = wt2[:, 0, 1]
    w['wt2AB'] = w2ab
    bias = np.zeros((D, 6), f)
    bias[:, 0] = np.asarray(inputs['enc_b1'], f)
    bias[:, 1] = np.asarray(inputs['enc_b2'], f)
    bias[:, 2] = np.asarray(inputs['enc_b3'], f)
    bias[:, 3] = np.asarray(inputs['dec_b0'], f)
    bias[:, 4] = np.asarray(inputs['dec_bt1'], f)
    bias[:, 5] = np.asarray(inputs['dec_bt2'], f)[0]
    w['biases'] = bias
    masks = np.ones((D, 2), f)                                        # ones vectors
    w['masks'] = masks
    emb = np.ascontiguousarray(np.asarray(inputs['embedding'], f))
    w['embedding'] = emb
    nrm = np.sqrt(np.sum(emb.astype(f) * emb, axis=-1, dtype=f)).astype(f)
    nrm = np.maximum(nrm, f(1e-12))
    w['embTn'] = np.ascontiguousarray(((emb / nrm[:, None]).astype(f)).T)
    return w


def _core_slabs(x, core):
    h = core % 2
    b = core // 2
    VQ_LO = 4096 * h - HALO
    t0 = 16 * VQ_LO - 26
    slab = np.zeros(LX, np.float32)
    lo, hi = max(0, t0), min(T, t0 + LX)
    if hi > lo:
        slab[lo - t0:hi - t0] = x[b, 0, lo:hi]
    x8 = np.zeros((8, L1P), np.float32)
    for t in range(8):
        x8[t] = slab[t:t + 4 * L1P:4]
    T0 = 65536 * h
    x4 = np.asarray(x[b, 0, T0:T0 + 65536], np.float32).reshape(16384, 4).T
    x4 = np.ascontiguousarray(x4)                                     # [4,16384]
    return x8, x4


# ------------------------------------------------------- custom DVE argmax
_ARGMAX_OP = None


def _get_argmax_op():
    global _ARGMAX_OP
    if _ARGMAX_OP is not None:
        return _ARGMAX_OP
    from concourse import dve_ops
    from concourse.dve_spec import (Spec, Src0, C0, Zero, MaxNeg, Idx, AluOp,
                                    lower as dve_lower, select, _has_src1)
    from concourse.dve_uop import DveOpSpec

    name = "ARGMAX_GE_ANT"
    if name in dve_ops._SUB_OPCODE_FOR_NAME:
        _ARGMAX_OP = next(op for op in dve_ops.OPS if op.name == name)
        return _ARGMAX_OP

    def _ref(in0, in1, c0, c1, c2):
        n = in0.shape[-1]
        iota = -np.arange(n, dtype=np.float32)
        out = np.where(in0.reshape(in0.shape[0], -1) >= c0, iota,
                       np.float32(-3.4028235e38))
        return out, out.max(-1, keepdims=True)

    spec = Spec(body=select(Src0 >= C0, Zero - Idx, MaxNeg),
                accum=AluOp.MAX, reference=_ref)
    row = max(dve_ops._SUB_OPCODE_FOR_NAME.values()) + 1
    shas = {}
    for ver in ("v3", "v4"):
        try:
            uops = dve_lower(spec, ver=ver)
            shas[ver] = DveOpSpec(name=name, opcode=row, uops=uops,
                                  rd1_en=_has_src1(spec)).sha(ver)
        except Exception:
            pass
    op = dve_ops.DveOp(name, spec, subdim=False, uops_sha=shas)
    dve_ops.OPS.append(op)
    dve_ops.CUSTOM_DVE_SPECS[name] = spec
    dve_ops._SUB_OPCODE_FOR_NAME[name] = row
    _ARGMAX_OP = op
    return op


# ------------------------------------------------------------ bass builder
_BUILT = None

# float32r streams at full PE rate but is reduced precision and must be
# produced (rounded) by the writing op; float32 is exact but 4 cycles/row.
# Flags flip buffer/weight dtypes per stage once precision is measured.
DEC_F32R = True
PACK_CONV1 = False


def _build():
    global _BUILT, F32
    if _BUILT is not None:
        return _BUILT
    import concourse.bass as bass
    import concourse.mybir as mybir
    import concourse.tile as tile
    from concourse import bacc
    from concourse.masks import make_identity
    from contextlib import ExitStack

    F32 = mybir.dt.float32
    F32R = mybir.dt.float32r
    AF = mybir.ActivationFunctionType
    ALU = mybir.AluOpType
    AX = mybir.AxisListType
    argmax_op = _get_argmax_op()

    nc = bacc.Bacc("TRN2", target_bir_lowering=False, debug=False,
                   enable_asserts=False, num_devices=8)

    dt_i32 = mybir.dt.int32
    # ---- DRAM tensors
    x8_d = nc.dram_tensor("x8", [8, L1P], F32, kind="ExternalInput").ap()
    x4_d = nc.dram_tensor("x4", [4, 16384], F32, kind="ExternalInput").ap()
    w1_d = nc.dram_tensor("w1T", [D, D], F32, kind="ExternalInput").ap()
    w2_d = nc.dram_tensor("w2T", [D, 8, D], F32, kind="ExternalInput").ap()
    w3_d = nc.dram_tensor("w3T", [D, 3, D], F32, kind="ExternalInput").ap()
    w0_d = nc.dram_tensor("w0T", [D, 3, D], F32, kind="ExternalInput").ap()
    wt1_d = nc.dram_tensor("wt1AB", [D, 8, D], F32, kind="ExternalInput").ap()
    wt2_d = nc.dram_tensor("wt2AB", [D, 12], F32, kind="ExternalInput").ap()
    bias_d = nc.dram_tensor("biases", [D, 6], F32, kind="ExternalInput").ap()
    mask_d = nc.dram_tensor("masks", [D, 2], F32, kind="ExternalInput").ap()
    emb_d = nc.dram_tensor("embedding", [KCB, D], F32, kind="ExternalInput").ap()
    embn_d = nc.dram_tensor("embTn", [D, KCB], F32, kind="ExternalInput").ap()
    xr_d = nc.dram_tensor("xr_out", [65536], F32, kind="ExternalOutput").ap()
    idx_d = nc.dram_tensor("idx_out", [4096], dt_i32, kind="ExternalOutput").ap()
    part_d = nc.dram_tensor("partials", [2, 2], F32, kind="ExternalOutput").ap()


    ctx = ExitStack()
    with tile.TileContext(nc) as tc, ctx:
        P = ctx.enter_context(tc.tile_pool(name="persist", bufs=1))
        ps = ctx.enter_context(tc.tile_pool(name="ps", bufs=1, space="PSUM"))
        sl = ctx.enter_context(tc.tile_pool(name="slabs", bufs=2))

        # ---- persistent SBUF
        w1sb = P.tile([D, D], F32, tag="w1")
        w2sb = P.tile([D, 8, D], F32, tag="w2")
        w3sb = P.tile([D, 3, D], F32, tag="w3")
        w0sb = P.tile([D, 3, D], F32, tag="w0")
        wt1sb = P.tile([D, 8, D], F32, tag="wt1")
        wt2sb = P.tile([D, 12], F32, tag="wt2")
        biassb = P.tile([D, 6], F32, tag="bias")
        masksb = P.tile([D, 2], F32, tag="mask")
        ident = P.tile([D, D], F32, tag="ident")
        embTn = P.tile([D, KCB], F32, tag="embTn")
        y2 = P.tile([D, L2 + 2], F32, tag="y2")
        z = P.tile([D, NPOS], F32, tag="z")
        zqg = P.tile([D, NT, D], F32, tag="zqg")
        DTD = F32R if DEC_F32R else F32
        zqT = P.tile([D, NPOS], DTD, tag="zqT")
        h0 = P.tile([D, NPOS], DTD, tag="h0")
        idx32 = P.tile([D, NT], dt_i32, tag="idx32")
        parts = P.tile([D, 48], F32, tag="parts")
        nc.gpsimd.memset(parts[:], 0.0)
        pairb = P.tile([D, 2], F32, tag="pairb")
        scr = P.tile([D, KCB], F32, tag="scr")


        for sb_t, d_ap in ((w1sb, w1_d), (w2sb, w2_d), (w3sb, w3_d),
                           (w0sb, w0_d), (wt1sb, wt1_d), (wt2sb, wt2_d),
                           (biassb, bias_d), (masksb, mask_d)):
            nc.sync.dma_start(sb_t[:], d_ap[:])
        make_identity(nc, ident[:])
        w0r = P.tile([D, 3, D], DTD, tag="w0r")
        wt1r = P.tile([D, 8, D], DTD, tag="wt1r")
        wt2r = P.tile([D, 12], DTD, tag="wt2r")
        nc.vector.tensor_copy(w0r[:], w0sb[:])
        nc.vector.tensor_copy(wt1r[:], wt1sb[:])
        nc.vector.tensor_copy(wt2r[:], wt2sb[:])

        def bias_ap(i):
            return biassb[:, i:i + 1]



        # ---------------------------------------------------- phase emitters
        def conv1_slab(c):
            lc2 = min(512, L2 - 512 * c)
            w = 4 * lc2 + 4
            n0 = 2048 * c
            xsl = sl.tile([D, 2052], F32, tag="x8sl")
            for g in range(4):
                nc.sync.dma_start(xsl[32 * g:32 * g + 8, :w], x8_d[:, n0:n0 + w])
            y1s = sl.tile([D, 2052], F32, tag="y1sl")
            off = 0
            while off < w:
                n = min(512, w - off)
                pm = ps.tile([D, 512], F32, tag="mm", bufs=2)
                if PACK_CONV1:
                    ng = (n + 127) // 128
                    for g in range(ng):
                        cn = min(128, n - 128 * g)
                        nc.tensor.matmul(pm[:, 128 * g:128 * g + cn],
                                         w1sb[32 * g:32 * g + 8, :],
                                         xsl[32 * g:32 * g + 8,
                                             off + 128 * g:off + 128 * g + cn],
                                         start=True, stop=True,
                                         tile_position=(32 * g, 0))
                else:
                    nc.tensor.matmul(pm[:, :n], w1sb[0:8, :],
                                     xsl[0:8, off:off + n],
                                     start=True, stop=True)
                nc.scalar.activation(y1s[:, off:off + n], pm[:, :n], AF.Gelu,
                                     bias=bias_ap(0))
                off += n
            return y1s

        def conv2_chunk(c, y1s):
            lc2 = min(512, L2 - 512 * c)
            pm = ps.tile([D, 512], F32, tag="mm", bufs=2)
            v3 = y1s[:].rearrange("p (n four) -> p n four", four=4)
            for t in range(8):
                rhs = v3[:, t // 4: t // 4 + lc2, t % 4]
                nc.tensor.matmul(pm[:, :lc2], w2sb[:, t, :], rhs,
                                 start=(t == 0), stop=(t == 7))
            nc.scalar.activation(y2[:, 512 * c:512 * c + lc2], pm[:, :lc2],
                                 AF.Gelu, bias=bias_ap(1))

        def conv3_chunk(c):
            zc = ZC[c]
            pm = ps.tile([D, 512], F32, tag="mm", bufs=2)
            for t in range(3):
                rhs = y2[:, 512 * c + t:512 * c + t + zc]
                nc.tensor.matmul(pm[:, :zc], w3sb[:, t, :], rhs,
                                 start=(t == 0), stop=(t == 2))
            nc.scalar.activation(z[:, 512 * c:512 * c + zc], pm[:, :zc],
                                 AF.Identity, bias=bias_ap(2))

        def vq_tile(i):
            simps = ps.tile([D, KCB], F32, tag="sim", bufs=1)
            lhsT = z[:, D * i:D * (i + 1)]
            for kk in range(4):
                nc.tensor.matmul(simps[:, 512 * kk:512 * (kk + 1)], lhsT,
                                 embTn[:, 512 * kk:512 * (kk + 1)],
                                 start=True, stop=True)
            mx = sl.tile([D, 1], F32, tag="mx")
            nc.vector.tensor_reduce(mx[:], simps[:], axis=AX.X, op=ALU.max)
            neg = sl.tile([D, 1], F32, tag="neg")
            nc.vector._custom_dve(argmax_op, out=scr[:], in0=simps[:],
                                  s0=mx[:], accum_out=neg[:])
            nc.vector.tensor_scalar(idx32[:, i:i + 1], neg[:], -1.0, None,
                                    op0=ALU.mult)
            nc.gpsimd.indirect_dma_start(
                out=zqg[:, i, :], out_offset=None, in_=emb_d[:],
                in_offset=bass.IndirectOffsetOnAxis(ap=idx32[:, i:i + 1], axis=0),
                bounds_check=KCB - 1, oob_is_err=False)
            tp = ps.tile([D, D], F32, tag="tp", bufs=1)
            nc.tensor.transpose(tp[:], zqg[:, i, :], ident[:])
            nc.scalar.copy(zqT[:, D * i:D * (i + 1)], tp[:])

        def com_chunk(r):
            c0 = HALO + 512 * r
            dsc = sl.tile([D, 512], F32, tag="dsc")
            nc.vector.tensor_tensor(out=dsc[:], in0=z[:, c0:c0 + 512],
                                    in1=zqT[:, c0:c0 + 512].bitcast(F32), op=ALU.subtract)
            nc.scalar.activation(dsc[:], dsc[:], AF.Square,
                                 accum_out=parts[:, r:r + 1])

        def conv0_chunk(cc):
            l0 = 62 + 512 * cc
            n = min(512, 4162 - l0)
            if n <= 0:
                return
            pm = ps.tile([D, 512], F32, tag="mm", bufs=2)
            for t in range(3):
                rhs = zqT[:, l0 - 1 + t:l0 - 1 + t + n]
                nc.tensor.matmul(pm[:, :n], w0r[:, t, :], rhs,
                                 start=(t == 0), stop=(t == 2))
            nc.scalar.activation(h0[:, l0:l0 + n], pm[:, :n], AF.Gelu,
                                 bias=bias_ap(3))

        def convT1_chunk(ct):
            """h1 slab [D, 4*514] m-contiguous, m0 = 4*U0, U0 = 63+512*ct."""
            U0 = 63 + 512 * ct
            h1s = sl.tile([D, 4 * 514], DTD, tag="h1sl")
            v3 = h1s[:].rearrange("p (u four) -> p u four", four=4)
            for r in range(4):
                dshift = -1 if r < 2 else 1
                for seg0, segn in ((0, 512), (512, 2)):
                    pm = ps.tile([D, 512], F32, tag="mm", bufs=2)
                    ra = h0[:, U0 + seg0:U0 + seg0 + segn]
                    rb = h0[:, U0 + dshift + seg0:U0 + dshift + seg0 + segn]
                    nc.tensor.matmul(pm[:, :segn], wt1r[:, r, :],
                                     ra, start=True, stop=False)
                    nc.tensor.matmul(pm[:, :segn], wt1r[:, 4 + r, :],
                                     rb, start=False, stop=True)
                    nc.scalar.activation(v3[:, seg0:seg0 + segn, r], pm[:, :segn],
                                         AF.Gelu, bias=bias_ap(4))
            return h1s

        def convT2_chunk(ct, h1s):
            for s in range(4):
                base = 4 + 512 * s
                pm = ps.tile([D, 512], F32, tag="mm", bufs=2)
                nc.tensor.matmul(pm[0:4, :], wt2r[:, 0:4],
                                 h1s[:, base:base + 512],
                                 start=True, stop=False)
                nc.tensor.matmul(pm[0:4, :], wt2r[:, 4:8],
                                 h1s[:, base - 1:base + 511],
                                 start=False, stop=False)
                nc.tensor.matmul(pm[0:4, :], wt2r[:, 8:12],
                                 h1s[:, base + 1:base + 513],
                                 start=False, stop=True)
                xrt = sl.tile([4, 512], F32, tag="xrt")
                nc.scalar.activation(xrt[:], pm[0:4, :], AF.Tanh,
                                     bias=biassb[0:4, 5:6])
                o = 8192 * ct + 2048 * s
                nc.sync.dma_start(
                    xr_d[o:o + 2048].rearrange("(i four) -> four i", four=4),
                    xrt[:])
                xc = sl.tile([4, 512], F32, tag="xcmp")
                nc.sync.dma_start(
                    xc[:], x4_d[:, 2048 * ct + 512 * s:2048 * ct + 512 * (s + 1)])
                dsc = sl.tile([4, 512], F32, tag="dsc2")
                nc.vector.tensor_tensor(out=dsc[:], in0=xrt[:], in1=xc[:],
                                        op=ALU.subtract)
                nc.scalar.activation(dsc[:], dsc[:], AF.Square,
                                     accum_out=parts[0:4, 9 + 4 * ct + s:10 + 4 * ct + s])

        # ------------------------------------------------------ emission
        # conv3(c) reads 2 y2 columns produced by conv2(c+1); com/conv0 read
        # one z/zqT tile of the next VQ chunk; convT1 reads 3 h0 columns of
        # the next conv0 chunk -- so each stage lags its producer by one
        # chunk in emission order.
        for c in range(NCHUNK + 3):
            if c < NCHUNK:
                y1s = conv1_slab(c)
            if c == 0:
                nc.sync.dma_start(embTn[:], embn_d[:])
            if c < NCHUNK:
                conv2_chunk(c, y1s)
            if 1 <= c <= NCHUNK:
                conv3_chunk(c - 1)
                t0 = 4 * (c - 1)
                for i in range(t0, min(NT, t0 + 4)):
                    vq_tile(i)
            if 2 <= c:
                if c - 2 < 8:
                    com_chunk(c - 2)
                if c - 2 <= 8:
                    conv0_chunk(c - 2)
            if 3 <= c and c - 3 < 8:
                h1s = convT1_chunk(c - 3)
                convT2_chunk(c - 3, h1s)

        # ------------------------------------------------------ epilogue
        nc.vector.tensor_reduce(pairb[:, 0:1], parts[:, 9:41], axis=AX.X,
                                op=ALU.add)
        nc.vector.tensor_reduce(pairb[:, 1:2], parts[:, 0:8], axis=AX.X,
                                op=ALU.add)
        pp = ps.tile([2, 2], F32, tag="mm", bufs=2)
        nc.tensor.matmul(pp[:], masksb[:], pairb[:], start=True, stop=True)
        ppsb = P.tile([2, 2], F32, tag="ppsb")
        nc.scalar.copy(ppsb[:], pp[:])
        nc.sync.dma_start(part_d[:], ppsb[:])

        # idx output: interior positions 64..4159
        nc.sync.dma_start(idx_d[0:64], idx32[64:128, 0:1])
        nc.sync.dma_start(idx_d[64:64 + 31 * 128].rearrange("(t p) -> p t", p=128),
                          idx32[:, 1:32])
        nc.sync.dma_start(idx_d[4032:4096], idx32[0:64, 32:33])

    nc.compile()
    _BUILT = nc
    return nc


# ---------------------------------------------------------------- kernel()
LAST_RUN = None


_CACHE_CLEARED = False


def _clear_stale_neff_cache():
    global _CACHE_CLEARED
    if _CACHE_CLEARED:
        return
    _CACHE_CLEARED = True
    import shutil
    for p in (os.path.expanduser("~/.neuron-compile-cache"),):
        if os.path.isdir(p):
            shutil.rmtree(p, ignore_errors=True)


def kernel(**inputs):
    from concourse.bass_utils import run_bass_kernel_spmd
    global LAST_RUN
    _clear_stale_neff_cache()

    w = _host_prepare(inputs)
    x = np.asarray(inputs['x'], np.float32)
    shared = {
        'w1T': w['w1T'], 'w2T': w['w2T'], 'w3T': w['w3T'], 'w0T': w['w0T'],
        'wt1AB': w['wt1AB'], 'wt2AB': w['wt2AB'], 'biases': w['biases'],
        'masks': w['masks'], 'embedding': w['embedding'], 'embTn': w['embTn'],
    }
    in_maps = []
    for core in range(8):
        x8, x4 = _core_slabs(x, core)
        m = dict(shared)
        m['x8'] = x8
        m['x4'] = x4
        in_maps.append(m)

    nc = _build()
    res = run_bass_kernel_spmd(nc, in_maps, core_ids=list(range(8)))
    LAST_RUN = res

    xr_full = np.zeros((B, 1, T), np.float32)
    idx_full = np.zeros((B, 8192), np.int32)
    rec_sum = 0.0
    com_sum = 0.0
    for core in range(8):
        r = res.results[core]
        b, h = core // 2, core % 2
        xr_full[b, 0, 65536 * h:65536 * (h + 1)] = r['xr_out']
        idx_full[b, 4096 * h:4096 * (h + 1)] = r['idx_out']
        rec_sum += float(r['partials'][0, 0])
        com_sum += float(r['partials'][1, 1])

    rec = np.float32(rec_sum / (B * T))
    com = np.float32(com_sum / (B * D * 8192))
    cs = np.asarray(inputs['cluster_size'], np.float64)
    n = cs / (cs.sum() + 1e-6)
    ppx = np.float32(np.exp(-np.sum(n * np.log(n + 1e-6))))
    return rec, com, ppx, xr_full, idx_full
